# revision 1
# baseline (speedup 1.0000x reference)
"""Trainium2 Bass kernel for the HNX scatter-memory block.

Sharding: 8 cores = (batch b in 0..3) x (sequence half j in 0..1).
Each core processes its 1024-token window plus W warmup tokens on each
side (zero-padded + masked at sequence edges), so both the forward and
backward EMA scans converge to the exact state before the window starts
(truncation error ~ sigmoid(decay)^W).  No inter-core communication.

On-chip layout is "scan layout": channels on partitions, time along the
free dimension.  x is pre-transposed on the host.  The EMA recurrences
use the native DVE tensor_tensor_scan.  Channel-dim reductions
(softmax-entropy, slot logits) are PE matmuls against ones / weight
columns; per-token scalars are re-broadcast across partitions with
gpsimd.partition_broadcast.
"""

import numpy as np
from contextlib import ExitStack

import concourse.bacc as bacc
import concourse.tile as tile
from concourse import mybir
from concourse.bass_utils import run_bass_kernel_spmd

F32 = mybir.dt.float32
AF = mybir.ActivationFunctionType
OP = mybir.AluOpType


class Cfg:
    def __init__(self, DI=1024, H=1024, O=1024, S=128, T=2048, W=512, CH=512,
                 mm_dtype=F32, sim_acts=False):
        self.DI, self.H, self.O, self.S, self.T, self.W, self.CH = DI, H, O, S, T, W, CH
        self.Tout = T // 2            # tokens per core window
        self.Tw = self.Tout + 2 * W   # work tokens per core
        self.KG = DI // 128           # input k-tiles
        self.HG = H // 128            # hidden channel groups
        self.OG = O // 128            # output channel groups
        self.NCH = self.Tw // CH      # phase-1 chunks
        self.WCH = self.Tout // CH    # phase-3 (window) chunks
        self.mm_dtype = mm_dtype
        self.sim_acts = sim_acts
        assert self.S == 128 and self.Tw % CH == 0 and self.Tout % CH == 0
        assert CH <= 512 and self.W <= CH


# chp column layout: per-channel params, one column per (param, group)
CHP_NAMES = ["k0", "k1", "omdf", "df", "omdb", "db", "sbias", "ba"]
SC_F1, SC_F0, SC_NF1, SC_F2, SC_NSW, SC_NSB, SC_NBMG = range(7)


def build_program(cfg: Cfg):
    c = cfg
    nc = bacc.Bacc("TRN2", target_bir_lowering=False, debug=False,
                   enable_asserts=False)

    def mm(ap):
        return ap if c.mm_dtype == F32 else ap.bitcast(c.mm_dtype)

    xt = nc.dram_tensor("xt", [c.DI, c.Tw], F32, kind="ExternalInput").ap()
    w_in = nc.dram_tensor("w_in", [c.DI, 2 * c.H], F32, kind="ExternalInput").ap()
    w_out = nc.dram_tensor("w_out", [c.H, c.O], F32, kind="ExternalInput").ap()
    w_slot = nc.dram_tensor("w_slot", [c.H, c.S], F32, kind="ExternalInput").ap()
    w_mg = nc.dram_tensor("w_mg", [c.H, 1], F32, kind="ExternalInput").ap()
    mem_bank = nc.dram_tensor("mem_bank", [c.S, c.H], F32, kind="ExternalInput").ap()
    chp = nc.dram_tensor("chp", [128, len(CHP_NAMES) * c.HG], F32,
                         kind="ExternalInput").ap()
    bout_d = nc.dram_tensor("bout", [128, c.OG], F32, kind="ExternalInput").ap()
    bslot_d = nc.dram_tensor("bslot", [128, 1], F32, kind="ExternalInput").ap()
    sc = nc.dram_tensor("sc", [1, 8], F32, kind="ExternalInput").ap()
    mask_d = nc.dram_tensor("mask", [1, c.Tw], F32, kind="ExternalInput").ap()
    out_d = nc.dram_tensor("outT", [c.O, c.Tout], F32, kind="ExternalOutput").ap()

    with tile.TileContext(nc) as tc:
        with ExitStack() as top:
            consts = top.enter_context(tc.tile_pool(name="consts", bufs=1))
            chp_t = consts.tile([128, len(CHP_NAMES) * c.HG], F32)
            nc.sync.dma_start(chp_t[:], chp[:])
            sc_t = consts.tile([1, 8], F32)
            nc.sync.dma_start(sc_t[:], sc[:])
            bout_t = consts.tile([128, c.OG], F32)
            nc.sync.dma_start(bout_t[:], bout_d[:])
            bslot_t = consts.tile([128, 1], F32)
            nc.sync.dma_start(bslot_t[:], bslot_d[:])
            ones_t = consts.tile([128, 1], F32)
            nc.vector.memset(ones_t[:], 1.0)
            mb = consts.tile([128, c.Tw], F32)
            nc.sync.dma_start(mb[:], mask_d.broadcast_to([128, c.Tw]))
            wslot_t = [consts.tile([128, c.S], F32, name=f"ws{k}", tag=f"ws{k}")
                       for k in range(c.HG)]
            for k in range(c.HG):
                nc.sync.dma_start(wslot_t[k][:], w_slot[k * 128:(k + 1) * 128, :])
            wmg_t = [consts.tile([128, 1], F32, name=f"wmg{k}", tag=f"wmg{k}")
                     for k in range(c.HG)]
            for k in range(c.HG):
                nc.sync.dma_start(wmg_t[k][:], w_mg[k * 128:(k + 1) * 128, :])
            memb_t = consts.tile([128, c.H], F32)
            nc.sync.dma_start(memb_t[:], mem_bank[:])

            def chpc(name, g):
                i = CHP_NAMES.index(name) * c.HG + g
                return chp_t[:, i:i + 1]

            def scc(i):
                return sc_t[0:1, i:i + 1]

            fpool = top.enter_context(tc.tile_pool(name="f", bufs=1))
            f_t = [fpool.tile([128, c.Tw - c.W], F32, name=f"f{g}", tag=f"f{g}")
                   for g in range(c.HG)]

            # ---------------- phase 1: in-proj, conv, fwd scan ----------
            with ExitStack() as p1:
                xt_pool = p1.enter_context(tc.tile_pool(name="xt", bufs=1))
                xt_t = [xt_pool.tile([128, c.Tw], F32, name=f"xt{k}", tag=f"xt{k}")
                        for k in range(c.KG)]
                for k in range(c.KG):
                    nc.sync.dma_start(xt_t[k][:], xt[k * 128:(k + 1) * 128, :])

                win = p1.enter_context(tc.tile_pool(name="win", bufs=2))
                pch = p1.enter_context(tc.tile_pool(name="pch", bufs=2))
                scr = p1.enter_context(tc.tile_pool(name="scr", bufs=2))
                ps1 = p1.enter_context(tc.tile_pool(name="ps1", bufs=2,
                                                    space="PSUM"))

                for g in range(c.HG):
                    wa = win.tile([128, 128 * c.KG], F32, tag="wa")
                    nc.sync.dma_start(
                        wa[:].rearrange("p (k m) -> p k m", m=128),
                        w_in[:, g * 128:(g + 1) * 128]
                        .rearrange("(k p) m -> p k m", p=128))
                    wd = win.tile([128, 128 * c.KG], F32, tag="wd")
                    nc.sync.dma_start(
                        wd[:].rearrange("p (k m) -> p k m", m=128),
                        w_in[:, c.H + g * 128:c.H + (g + 1) * 128]
                        .rearrange("(k p) m -> p k m", p=128))

                    x1 = pch.tile([128, c.Tw], F32, tag="x1")
                    for n in range(c.NCH):
                        sl = slice(n * c.CH, (n + 1) * c.CH)
                        pa = ps1.tile([128, c.CH], F32, tag="pa")
                        pdt = ps1.tile([128, c.CH], F32, tag="pdt")
                        for k in range(c.KG):
                            nc.tensor.matmul(
                                pa[:], mm(wa[:, k * 128:(k + 1) * 128]),
                                mm(xt_t[k][:, sl]),
                                start=(k == 0), stop=(k == c.KG - 1))
                        for k in range(c.KG):
                            nc.tensor.matmul(
                                pdt[:], mm(wd[:, k * 128:(k + 1) * 128]),
                                mm(xt_t[k][:, sl]),
                                start=(k == 0), stop=(k == c.KG - 1))
                        sdt = scr.tile([128, c.CH], F32, tag="sdt")
                        if c.sim_acts:
                            nc.scalar.activation(sdt[:], pdt[:], AF.Sigmoid,
                                                 bias=chpc("sbias", g))
                            nc.vector.scalar_tensor_tensor(
                                sdt[:], pdt[:], chpc("sbias", g), sdt[:],
                                OP.add, OP.mult)
                        else:
                            nc.scalar.activation(sdt[:], pdt[:], AF.Silu,
                                                 bias=chpc("sbias", g))
                        if n == 0:
                            # zero the left-pad region so the causal conv
                            # sees x_prev=0 at the true sequence start
                            nc.vector.tensor_tensor(sdt[:], sdt[:],
                                                    mb[:, sl], OP.mult)
                        # x1 = (a + b_a) * silu(dt + sbias)
                        nc.vector.scalar_tensor_tensor(
                            x1[:, sl], pa[:], chpc("ba", g), sdt[:],
                            OP.add, OP.mult)

                    # causal depthwise conv k=2 + silu
                    tmp = pch.tile([128, c.Tw], F32, tag="ta")
                    nc.gpsimd.memset(tmp[:, 0:1], 0.0)
                    nc.scalar.activation(tmp[:, 1:c.Tw], x1[:, 0:c.Tw - 1],
                                         AF.Identity, scale=chpc("k0", g))
                    ypre = pch.tile([128, c.Tw], F32, tag="tb")
                    nc.vector.scalar_tensor_tensor(
                        ypre[:], x1[:], chpc("k1", g), tmp[:], OP.mult, OP.add)
                    ysl = pch.tile([128, c.Tw], F32, tag="x1")
                    if c.sim_acts:
                        nc.scalar.activation(ysl[:], ypre[:], AF.Sigmoid)
                        nc.vector.tensor_tensor(ysl[:], ypre[:], ysl[:],
                                                OP.mult)
                    else:
                        nc.scalar.activation(ysl[:], ypre[:], AF.Silu)
                    u = pch.tile([128, c.Tw], F32, tag="ta")
                    nc.vector.scalar_tensor_tensor(
                        u[:], ysl[:], chpc("omdf", g), mb[:], OP.mult, OP.mult)

                    # fwd EMA scan; first W tokens into discard scratch
                    dfb_w = chpc("df", g).broadcast_to([128, c.W])
                    dfb_m = chpc("df", g).broadcast_to([128, c.Tw - c.W])
                    fscr = scr.tile([128, c.W], F32, tag="sdt")
                    nc.vector.tensor_tensor_scan(
                        fscr[:], dfb_w, u[:, 0:c.W], 0.0, OP.mult, OP.add)
                    nc.vector.tensor_tensor_scan(
                        f_t[g][:], dfb_m, u[:, c.W:c.Tw],
                        fscr[:, c.W - 1:c.W], OP.mult, OP.add)

            # ------------- phase 2+3: bwd scan, memory, fusion, out ------
            with ExitStack() as p2:
                gpool = p2.enter_context(tc.tile_pool(name="gb", bufs=1))
                gb_t = [gpool.tile([128, c.Tout], F32, name=f"gb{g}", tag=f"gb{g}")
                        for g in range(c.HG)]

                p2s = p2.enter_context(tc.tile_pool(name="p2s", bufs=1))
                for g in range(c.HG):
                    Lw = c.Tw - c.W
                    d1 = p2s.tile([128, Lw], F32, tag="d1")
                    nc.vector.scalar_tensor_tensor(
                        d1[:], f_t[g][:], chpc("omdb", g), mb[:, c.W:c.Tw],
                        OP.mult, OP.mult)
                    dbb_w = chpc("db", g).broadcast_to([128, c.W])
                    dbb_m = chpc("db", g).broadcast_to([128, c.Tout])
                    bscr = p2s.tile([128, c.W], F32, tag="bscr")
                    nc.vector.tensor_tensor_scan(
                        bscr[:, ::-1], dbb_w, d1[:, c.Tout:Lw][:, ::-1],
                        0.0, OP.mult, OP.add)
                    nc.vector.tensor_tensor_scan(
                        gb_t[g][:, ::-1], dbb_m, d1[:, 0:c.Tout][:, ::-1],
                        bscr[:, 0:1], OP.mult, OP.add)

                p3 = p2.enter_context(tc.tile_pool(name="p3", bufs=2))
                pb1 = p2.enter_context(tc.tile_pool(name="pb1", bufs=1))
                wpool = p2.enter_context(tc.tile_pool(name="wp", bufs=1))
                wo_pool = p2.enter_context(tc.tile_pool(name="wo", bufs=3))
                row = p2.enter_context(tc.tile_pool(name="row", bufs=1))
                psS = p2.enter_context(tc.tile_pool(name="psS", bufs=1,
                                                    space="PSUM"))
                psR = p2.enter_context(tc.tile_pool(name="psR", bufs=1,
                                                    space="PSUM"))
                psM = p2.enter_context(tc.tile_pool(name="psM", bufs=2,
                                                    space="PSUM"))
                psO = p2.enter_context(tc.tile_pool(name="psO", bufs=2,
                                                    space="PSUM"))

                for w in range(c.WCH):
                    sl = slice(w * c.CH, (w + 1) * c.CH)
                    pZ = psR.tile([1, c.CH], F32, tag="pZ")
                    pG = psR.tile([1, c.CH], F32, tag="pG")
                    pM = psR.tile([1, c.CH], F32, tag="pM")
                    pL = psS.tile([128, c.CH], F32, tag="pL")
                    for g in range(c.HG):
                        st, sp = (g == 0), (g == c.HG - 1)
                        pt = p3.tile([128, c.CH], F32, tag="p")
                        nc.scalar.activation(pt[:], f_t[g][:, sl], AF.Exp)
                        pft = p3.tile([128, c.CH], F32, tag="pf")
                        nc.vector.tensor_tensor(pft[:], pt[:], f_t[g][:, sl],
                                                OP.mult)
                        nc.tensor.matmul(pZ[:], mm(ones_t[:]),
                                         mm(pt[:]), start=st, stop=sp)
                        nc.tensor.matmul(pG[:], mm(ones_t[:]),
                                         mm(pft[:]), start=st, stop=sp)
                        nc.tensor.matmul(pM[:], mm(wmg_t[g][:]),
                                         mm(f_t[g][:, sl]), start=st, stop=sp)
                        nc.tensor.matmul(pL[:], mm(wslot_t[g][:]),
                                         mm(f_t[g][:, sl]), start=st, stop=sp)

                    E = p3.tile([128, c.CH], F32, tag="E")
                    nc.scalar.activation(E[:], pL[:], AF.Exp, bias=bslot_t[:])
                    # reuse row 0 of the (now dead) slot-logit bank for Zs
                    pZs = pL[0:1, :]
                    nc.tensor.matmul(pZs, mm(ones_t[:]), mm(E[:]),
                                     start=True, stop=True)

                    # per-token gate scalars ([1, CH] rows)
                    Zr = row.tile([1, c.CH], F32, tag="Zr")
                    nc.vector.reciprocal(Zr[:], pZ[:])
                    lnZ = row.tile([1, c.CH], F32, tag="lnZ")
                    nc.scalar.activation(lnZ[:], pZ[:], AF.Ln)
                    gz = row.tile([1, c.CH], F32, tag="gz")
                    nc.vector.tensor_tensor(gz[:], pG[:], Zr[:], OP.mult)
                    ent = row.tile([1, c.CH], F32, tag="ent")
                    nc.vector.tensor_tensor(ent[:], lnZ[:], gz[:], OP.subtract)
                    sg = row.tile([1, c.CH], F32, tag="sg")
                    nc.scalar.activation(sg[:], ent[:], AF.Exp,
                                         scale=scc(SC_NSW), bias=scc(SC_NSB))
                    sg1 = row.tile([1, c.CH], F32, tag="sg1")
                    nc.vector.tensor_scalar(sg1[:], sg[:], 1.0, None, OP.add)
                    gate = row.tile([1, c.CH], F32, tag="gate")
                    nc.vector.reciprocal(gate[:], sg1[:])
                    A = row.tile([1, c.CH], F32, tag="A")
                    nc.vector.tensor_scalar(A[:], gate[:], scc(SC_F1),
                                            scc(SC_F0), OP.mult, OP.add)
                    B = row.tile([1, c.CH], F32, tag="B")
                    nc.vector.tensor_scalar(B[:], gate[:], scc(SC_NF1), None,
                                            OP.mult)
                    mgs = row.tile([1, c.CH], F32, tag="mgs")
                    nc.scalar.activation(mgs[:], pM[:], AF.Exp,
                                         scale=-1.0, bias=scc(SC_NBMG))
                    mg1 = row.tile([1, c.CH], F32, tag="mg1")
                    nc.vector.tensor_scalar(mg1[:], mgs[:], 1.0, None, OP.add)
                    mgi = row.tile([1, c.CH], F32, tag="mgi")
                    nc.vector.reciprocal(mgi[:], mg1[:])
                    Zsr = row.tile([1, c.CH], F32, tag="Zsr")
                    nc.vector.reciprocal(Zsr[:], pZs)
                    s2 = row.tile([1, c.CH], F32, tag="s2")
                    nc.vector.scalar_tensor_tensor(s2[:], mgi[:], scc(SC_F2),
                                                   Zsr[:], OP.mult, OP.mult)

                    AB = pb1.tile([128, c.CH], F32, tag="AB")
                    nc.gpsimd.partition_broadcast(AB[:], A[:])
                    BB = pb1.tile([128, c.CH], F32, tag="BB")
                    nc.gpsimd.partition_broadcast(BB[:], B[:])
                    S2B = pb1.tile([128, c.CH], F32, tag="S2B")
                    nc.gpsimd.partition_broadcast(S2B[:], s2[:])

                    E2 = p3.tile([128, c.CH], F32, tag="E")
                    nc.vector.tensor_tensor(E2[:], E[:], S2B[:], OP.mult)

                    # memory read + fusion -> weighted (scan layout)
                    w_t = []
                    for g in range(c.HG):
                        pm = psM.tile([128, c.CH], F32, tag="pm")
                        nc.tensor.matmul(
                            pm[:], mm(memb_t[:, g * 128:(g + 1) * 128]),
                            mm(E2[:]), start=True, stop=True)
                        t1 = p3.tile([128, c.CH], F32, tag="t1")
                        nc.vector.tensor_tensor(t1[:], f_t[g][:, sl], AB[:],
                                                OP.mult)
                        t2 = p3.tile([128, c.CH], F32, tag="t2")
                        nc.gpsimd.tensor_tensor(t2[:], gb_t[g][:, sl], BB[:],
                                                OP.mult)
                        t3 = p3.tile([128, c.CH], F32, tag="t3")
                        nc.vector.tensor_tensor(t3[:], t1[:], t2[:], OP.add)
                        wt = wpool.tile([128, c.CH], F32, name=f"w{g}", tag=f"w{g}")
                        nc.vector.tensor_tensor(wt[:], t3[:], pm[:], OP.add)
                        w_t.append(wt)

                    # out-proj
                    for m in range(c.OG):
                        po = psO.tile([128, c.CH], F32, tag="po")
                        for k in range(c.HG):
                            wok = wo_pool.tile([128, 128], F32, tag="wo")
                            nc.sync.dma_start(
                                wok[:], w_out[k * 128:(k + 1) * 128,
                                              m * 128:(m + 1) * 128])
                            nc.tensor.matmul(
                                po[:], mm(wok[:]), mm(w_t[k][:]),
                                start=(k == 0), stop=(k == c.HG - 1))
                        ob = p3.tile([128, c.CH], F32, tag="ob")
                        nc.scalar.activation(ob[:], po[:], AF.Identity,
                                             bias=bout_t[:, m:m + 1])
                        nc.sync.dma_start(out_d[m * 128:(m + 1) * 128, sl], ob[:])

    nc.compile()
    return nc


_PROG_CACHE = {}


def _get_prog(cfg: Cfg):
    key = (cfg.DI, cfg.H, cfg.O, cfg.S, cfg.T, cfg.W, cfg.CH, str(cfg.mm_dtype))
    if key not in _PROG_CACHE:
        _PROG_CACHE[key] = build_program(cfg)
    return _PROG_CACHE[key]


def make_in_maps(cfg, x, W_in, b_in, dt_bias_fwd, conv_k, decay_fwd, decay_bwd,
                 memory, mem_decay, W_mem_gate, b_mem_gate, W_slot, b_slot,
                 W_slot_bwd, b_slot_bwd, fusion_weight, scaler_w, scaler_b,
                 W_out, b_out):
    c = cfg
    x = np.asarray(x)
    B, T, DI = x.shape
    f32 = np.float32

    def sig(v):
        return 1.0 / (1.0 + np.exp(-np.asarray(v, np.float64)))

    def col(v):  # [H] -> [128, HG] column blocks
        return np.ascontiguousarray(np.asarray(v, f32).reshape(c.HG, 128).T)

    df = sig(decay_fwd)
    db = sig(decay_bwd)
    chp = np.concatenate([
        col(conv_k[:, 0]), col(conv_k[:, 1]),
        col((1.0 - df)), col(df),
        col((1.0 - db)), col(db),
        col(np.asarray(b_in)[c.H:] + np.asarray(dt_bias_fwd)),
        col(np.asarray(b_in)[:c.H]),
    ], axis=1).astype(f32)
    bout = np.ascontiguousarray(np.asarray(b_out, f32).reshape(c.OG, 128).T)
    bslot = np.asarray(b_slot_bwd, f32).reshape(128, 1)
    scv = np.zeros((1, 8), f32)
    scv[0, SC_F1] = fusion_weight[1]
    scv[0, SC_F0] = fusion_weight[0]
    scv[0, SC_NF1] = -fusion_weight[1]
    scv[0, SC_F2] = fusion_weight[2]
    scv[0, SC_NSW] = -scaler_w[0]
    scv[0, SC_NSB] = -scaler_b[0]
    scv[0, SC_NBMG] = -b_mem_gate[0]
    mem_bank = (np.asarray(memory) * sig(mem_decay)[:, None]).astype(f32)

    shared = {
        "w_in": np.ascontiguousarray(np.asarray(W_in, f32)),
        "w_out": np.ascontiguousarray(np.asarray(W_out, f32)),
        "w_slot": np.ascontiguousarray(np.asarray(W_slot_bwd, f32)),
        "w_mg": np.ascontiguousarray(np.asarray(W_mem_gate, f32)),
        "mem_bank": mem_bank,
        "chp": chp, "bout": bout, "bslot": bslot, "sc": scv,
    }
    in_maps = []
    for core in range(8):
        b, j = divmod(core, 2)
        start = j * c.Tout - c.W
        gs, ge = max(0, start), min(T, start + c.Tw)
        xt = np.zeros((c.DI, c.Tw), f32)
        xt[:, gs - start:ge - start] = x[b, gs:ge, :].T
        mask = np.zeros((1, c.Tw), f32)
        mask[0, gs - start:ge - start] = 1.0
        m = dict(shared)
        m["xt"] = xt
        m["mask"] = mask
        in_maps.append(m)
    return in_maps


def run(cfg, inputs, trace=False, tmpdir=None):
    nc = _get_prog(cfg)
    in_maps = make_in_maps(cfg, **inputs)
    res = run_bass_kernel_spmd(nc, in_maps, core_ids=list(range(8)),
                               trace=trace, tmpdir=tmpdir)
    B, T = np.asarray(inputs["x"]).shape[0], np.asarray(inputs["x"]).shape[1]
    out = np.empty((B, T, cfg.O), np.float32)
    for core in range(8):
        b, j = divmod(core, 2)
        out[b, j * cfg.Tout:(j + 1) * cfg.Tout, :] = res.results[core]["outT"].T
    return out, res


def kernel(**inputs):
    cfg = Cfg()
    out, _ = run(cfg, inputs)
    return out

